# revision 10
# baseline (speedup 1.0000x reference)
"""BitLinear (BitNet b1.58-style) kernel for Trainium2, 8-core SPMD.

Reference computation (fp32):
    scale_w = max(mean(|W|), EPS)                       # scalar over all of W
    dq_w    = clip(round(W / scale_w), -1, 1) * scale_w # ternary weight
    amax_t  = max(max_j |x[t, j]|, EPS)                 # per token
    s_t     = 127 / amax_t
    dq_x    = round(x * s_t) / s_t                      # 8-bit absmax act quant
    out     = dq_x @ dq_w.T + b

Device strategy (data-parallel over tokens, fp8 DoubleRow matmul):
  * x is flattened to [8192, 4096] tokens, sharded 8 ways (1024/core) and
    shipped as bf16 (the 2e-2 tolerance dwarfs bf16 rounding; verified by
    exact simulation of this kernel's arithmetic: rel-err 0.0155, and the
    on-device run reproduces the simulated maxdiff bit-for-bit).
  * q_x = round(x*s) in [-127,127].  The first KF8=24 k-tiles are stored
    as fp8e4 (e4m3 RNE - the only lossy step) and their matmuls run pairs
    of k-tiles with perf_mode=DoubleRow (2 fp8 weights/PE cell, K=256 per
    instruction) at ~1.44x bf16 throughput.  The last 8 k-tiles stay
    exact bf16 to buy error margin.  q_w in {-1,0,1} is exact in both.
  * x is quantized token-major (per-token scale = per-partition ACT scale
    with a fused +C RNE round), then moved to feature-major by xbar
    DMA-transpose (2-byte dtype) + a GPSIMD cast copy.  No PE transposes.
  * W ships as W.T, column-blocked in per-core rotated order, so each
    core's first block doubles as its 1/8 shard for the |W| mean.  A
    dummy 4-byte AllReduce is issued at t=0 to absorb the cross-core
    startup-skew barrier so the real mean AllReduce completes quickly.
  * Engine plan: DVE = mean reduces, amax chain, W quant clips, evict
    scales; ACT = x round, W scale pass + its HWDGE ring carries W loads
    and output stores; sync ring = x loads + xbar transposes; GPSIMD =
    collectives, fp8/bf16 cast copies, bias adds, 2 W-quarter loads.
  * Matmul order: block 0 pair-inner (so it can start while phase A is
    still producing token tiles); later blocks token-outer with DR/bf16
    interleaved (hides the DoubleRow LDWEIGHTS under bf16 matmuls) and
    eviction immediately after each token tile's accumulation.
"""

import numpy as np
import ml_dtypes

from concourse import bacc, bass_isa, mybir, tile
from concourse.bass_utils import run_bass_kernel_spmd

F32 = mybir.dt.float32
BF16 = mybir.dt.bfloat16
FP8 = mybir.dt.float8e4
AX = mybir.AxisListType
OP = mybir.AluOpType
AF = mybir.ActivationFunctionType
PM = mybir.MatmulPerfMode

EPS = 1e-6
QMAX = 127.0
C_MAGIC = 1.5 * 2.0**23  # fp32 RNE rounding constant

N_CORES = 8
B, S, D_IN, D_OUT = 4, 2048, 4096, 4096
T_FULL = B * S
T = T_FULL // N_CORES  # 1024 tokens per core
KT = D_IN // 128  # 32 contraction k-tiles
KF8 = 24  # k-tiles carried in fp8 (12 DoubleRow pairs)
KBF = KT - KF8  # k-tiles carried in bf16 (exact)
NP8 = KF8 // 2  # 12 fp8 pairs
NT = D_OUT // 512  # 8 output column blocks
MT = T // 128  # 8 token tiles


def build_bass():
    nc = bacc.Bacc(None)
    xs_d = nc.dram_tensor("xs", [T, D_IN], BF16, kind="ExternalInput")
    wtb_d = nc.dram_tensor("wtb", [NT * D_IN, 512], F32, kind="ExternalInput")
    b_d = nc.dram_tensor("bias", [1, D_OUT], F32, kind="ExternalInput")
    out_d = nc.dram_tensor("out", [T, D_OUT], F32, kind="ExternalOutput")

    def wslice(j, qi):
        # quarter qi of column-block j: [1024, 512] of wtb, k-major
        r0 = (j * 4 + qi) * 1024
        return wtb_d[r0 : r0 + 1024, :].rearrange("(a p) c -> p a c", p=128)

    with tile.TileContext(nc) as tc:
        with (
            tc.tile_pool(name="persist", bufs=1) as persist,
            tc.tile_pool(name="small", bufs=2) as small,
            tc.tile_pool(name="wpipe", bufs=4) as wpipe,
            tc.tile_pool(name="dram", bufs=1, space="DRAM") as dram,
        ):
            # persistent state
            qxT8 = persist.tile([128, KF8, T], FP8)
            qxTb = persist.tile([128, KBF, T], BF16)
            bb = persist.tile([128, D_OUT], F32)
            amax_e = persist.tile([128, MT], F32)  # max(amax, EPS) per token
            s_all = persist.tile([128, MT], F32)  # 127/amax per token
            c_all = persist.tile([128, MT], F32)  # amax*scale_w/127 per token
            scw = persist.tile([128, 1], F32)
            inv_w = persist.tile([128, 1], F32)

            # dummy collective at t=0: absorbs the cross-core startup-skew
            # barrier so the real AllReduce below is quick
            dum_in = dram.tile([1, 1], F32)
            dum_out = dram.tile([1, 1], F32, addr_space="Shared")
            nc.gpsimd.collective_compute(
                "AllReduce",
                OP.add,
                replica_groups=[list(range(N_CORES))],
                ins=[dum_in[:]],
                outs=[dum_out[:]],
            )

            # bias: load row 0 (sync), broadcast across partitions (gpsimd)
            nc.sync.dma_start(bb[0:1, :], b_d[:])
            nc.gpsimd.partition_broadcast(bb[:], bb[0:1, :], channels=128)

            # ---- |W| mean over this core's first column block ------------
            # eighth-sized loads on the ACT HWDGE ring, reduces on DVE
            wsum_p = small.tile([128, 8], F32)
            with tc.tile_pool(name="meanpipe", bufs=2) as meanpipe:
                for e in range(8):
                    mtl = meanpipe.tile([128, 4, 512], F32, tag="m_in")
                    nc.scalar.dma_start(
                        mtl[:],
                        wtb_d[e * 512 : (e + 1) * 512, :].rearrange(
                            "(a p) c -> p a c", p=128
                        ),
                    )
                    nc.vector.tensor_reduce(
                        out=wsum_p[:, e : e + 1],
                        in_=mtl[:],
                        axis=AX.XY,
                        op=OP.add,
                        apply_absolute_value=True,
                    )
            wsum1 = small.tile([128, 1], F32)
            nc.vector.tensor_reduce(out=wsum1[:], in_=wsum_p[:], axis=AX.X, op=OP.add)
            wsum_all = small.tile([128, 1], F32)
            nc.gpsimd.partition_all_reduce(
                wsum_all[:], wsum1[:], channels=128, reduce_op=bass_isa.ReduceOp.add
            )
            cc_in = dram.tile([1, 1], F32)
            cc_out = dram.tile([1, 1], F32, addr_space="Shared")
            nc.gpsimd.dma_start(cc_in[:], wsum_all[0:1, 0:1])
            nc.gpsimd.collective_compute(
                "AllReduce",
                OP.add,
                replica_groups=[list(range(N_CORES))],
                ins=[cc_in[:]],
                outs=[cc_out[:]],
            )

            # block-0 W quarters: 0/1 on the ACT ring (after the mean
            # eighths), 2/3 via GPSIMD SWDGE (after the collective issue)
            w0q = []
            for qi in range(4):
                wq = wpipe.tile([128, 8, 512], F32, tag="w_in", name="wq")
                eng = nc.scalar if qi < 2 else nc.gpsimd
                eng.dma_start(wq[:], wslice(0, qi))
                w0q.append(wq)

            # ---- phase A: x quant (token-major) + xbar transpose ---------
            with (
                tc.tile_pool(name="xpipe", bufs=2) as xpipe,
                tc.tile_pool(name="tpipe", bufs=2) as tpipe,
            ):
                for m in range(MT):
                    xtl = xpipe.tile([128, D_IN], BF16, tag="x_in")
                    nc.sync.dma_start(xtl[:], xs_d[m * 128 : (m + 1) * 128, :])
                    amax = xpipe.tile([128, 1], F32, tag="amax")
                    nc.vector.tensor_reduce(
                        out=amax[:],
                        in_=xtl[:],
                        axis=AX.X,
                        op=OP.max,
                        apply_absolute_value=True,
                    )
                    nc.vector.tensor_scalar(
                        amax_e[:, m : m + 1], amax[:], EPS, None, op0=OP.max
                    )
                    rec = xpipe.tile([128, 1], F32, tag="rec")
                    nc.vector.reciprocal(rec[:], amax_e[:, m : m + 1])
                    nc.vector.tensor_scalar(
                        s_all[:, m : m + 1], rec[:], QMAX, None, op0=OP.mult
                    )
                    # q = round(x*s) on ACT: v = x*s + C (RNE integer),
                    # then -C -> bf16 on DVE (q ints are bf16-exact)
                    qxc = xpipe.tile([128, D_IN], BF16, tag="qxc")
                    for h in range(2):
                        hs = slice(h * 2048, (h + 1) * 2048)
                        qq = xpipe.tile([128, 2048], F32, tag="qq")
                        nc.scalar.activation(
                            qq[:], xtl[:, hs], AF.Copy,
                            bias=C_MAGIC, scale=s_all[:, m : m + 1],
                        )
                        nc.vector.tensor_scalar(
                            qxc[:, hs], qq[:], C_MAGIC, None, op0=OP.subtract
                        )
                    # feature-major via xbar transpose on the sync ring,
                    # then cast copies on GPSIMD
                    t8 = tpipe.tile([128, KF8, 128], BF16, tag="t8")
                    nc.sync.dma_start_transpose(t8[:], qxc[:, 0 : KF8 * 128])
                    nc.gpsimd.tensor_copy(qxT8[:, :, m * 128 : (m + 1) * 128], t8[:])
                    tb = tpipe.tile([128, KBF, 128], BF16, tag="tb")
                    nc.sync.dma_start_transpose(tb[:], qxc[:, KF8 * 128 :])
                    nc.gpsimd.tensor_copy(qxTb[:, :, m * 128 : (m + 1) * 128], tb[:])

            # AllReduce readback (gpsimd, after the cast copies)
            tot = small.tile([1, 1], F32)
            nc.gpsimd.dma_start(tot[:], cc_out[:])
            tot_b = small.tile([128, 1], F32)
            nc.gpsimd.partition_broadcast(tot_b[:], tot[:], channels=128)

            # scale_w = max(total/(D_IN*D_OUT), EPS); inv_w = 1/scale_w
            nc.vector.tensor_scalar(
                scw[:], tot_b[:], 1.0 / (D_IN * D_OUT), EPS, op0=OP.mult, op1=OP.max
            )
            nc.vector.reciprocal(inv_w[:], scw[:])
            nc.vector.tensor_scalar(
                c_all[:], amax_e[:], scw[:, 0:1], 1.0 / QMAX, op0=OP.mult, op1=OP.mult
            )

            # ---- phase B: quantize W, matmul, evict ----------------------
            # Software pipelined: W-quant for block j+1 is emitted before
            # the matmuls of block j so its ACT/DVE work runs under them.
            qw8 = {}  # (j, kp) -> fp8 pair tile, kp in [0, NP8)
            qwb = {}  # (j, kq) -> bf16 pair tile, kq in [0, KBF//2)

            with (
                tc.tile_pool(name="qwpipe", bufs=1) as qwpipe,
                tc.tile_pool(name="opipe", bufs=5) as opipe,
                tc.tile_pool(name="psumB", bufs=1, space="PSUM") as psumB,
            ):

                def emit_wq(j, dma_tiles=()):
                    for qi in range(4):
                        if qi < len(dma_tiles):
                            wq = dma_tiles[qi]
                        else:
                            wq = wpipe.tile([128, 8, 512], F32, tag="w_in", name="wq")
                            nc.scalar.dma_start(wq[:], wslice(j, qi))
                        for pp in range(4):  # k-tile pairs within quarter
                            kp = qi * 4 + pp
                            pr = wq[:, 2 * pp : 2 * pp + 2, :]
                            # u = W*inv_w on ACT (separate op: the +C
                            # round then sees fl(W/scale) like the ref)
                            nc.scalar.activation(
                                pr, pr, AF.Copy, bias=0.0, scale=inv_w[:, 0:1]
                            )
                            # +C RNE round and clip in the C domain
                            nc.vector.tensor_scalar(
                                pr, pr, C_MAGIC, C_MAGIC + 1.0,
                                op0=OP.add, op1=OP.min,
                            )
                            if kp < NP8:
                                qt = qwpipe.tile(
                                    [128, 2, 512], FP8, tag=f"q8_{j % 2}", bufs=NP8
                                )
                                qw8[(j, kp)] = qt
                            else:
                                qt = qwpipe.tile(
                                    [128, 2, 512], BF16, tag=f"qb_{j % 2}", bufs=4
                                )
                                qwb[(j, kp - NP8)] = qt
                            nc.vector.tensor_scalar(
                                qt[:], pr, C_MAGIC - 1.0, C_MAGIC,
                                op0=OP.max, op1=OP.subtract,
                            )

                def mm_dr(ps, j, kp, m, start, stop=False):
                    nc.tensor.matmul(
                        ps[:],
                        qxT8[:, 2 * kp : 2 * kp + 2, m * 128 : (m + 1) * 128],
                        qw8[(j, kp)][:],
                        start=start,
                        stop=stop,
                        perf_mode=PM.DoubleRow,
                    )

                def mm_bf(ps, j, kb, m, stop, start=False):
                    nc.tensor.matmul(
                        ps[:],
                        qxTb[:, kb, m * 128 : (m + 1) * 128],
                        qwb[(j, kb // 2)][:, kb % 2, :],
                        start=start,
                        stop=stop,
                    )

                def emit_evict(j, m, ps):
                    ot = opipe.tile([128, 512], F32, tag="o_scaled")
                    nc.vector.tensor_scalar(
                        ot[:], ps[:], c_all[:, m : m + 1], None, op0=OP.mult
                    )
                    ot2 = opipe.tile([128, 512], F32, tag="o_final")
                    nc.gpsimd.tensor_tensor(
                        ot2[:], ot[:], bb[:, j * 512 : (j + 1) * 512], op=OP.add
                    )
                    nc.scalar.dma_start(
                        out_d[m * 128 : (m + 1) * 128, j * 512 : (j + 1) * 512],
                        ot2[:],
                    )

                def emit_mms(j):
                    pss = [None] * MT
                    for m in reversed(range(MT)):
                        pss[m] = psumB.tile([128, 512], F32, tag=f"mm{m}", name="ps")
                    if j == 0:
                        # pair-inner: consumes qw pairs at production rate
                        # and token tiles as phase A finishes them
                        for kp in range(NP8):
                            for m in range(MT):
                                mm_dr(pss[m], j, kp, m, start=(kp == 0))
                        for kb in range(KBF):
                            for m in range(MT):
                                mm_bf(pss[m], j, kb, m, stop=(kb == KBF - 1))
                        for m in range(MT):
                            emit_evict(j, m, pss[m])
                    else:
                        # token-outer with DR/bf16 interleave (bf16 FWL
                        # loads hide the DoubleRow LDWEIGHTS), eviction
                        # right after each token tile completes
                        for m in range(MT):
                            order = []
                            for k in range(KBF):
                                order += [("dr", k), ("bf", k)]
                            order += [("dr", k) for k in range(KBF, NP8)]
                            for i, (kind, k) in enumerate(order):
                                first = i == 0
                                last = i == len(order) - 1
                                if kind == "dr":
                                    mm_dr(pss[m], j, k, m, start=first, stop=last)
                                else:
                                    mm_bf(pss[m], j, k, m, stop=last, start=first)
                            emit_evict(j, m, pss[m])
                    for kp in range(NP8):
                        del qw8[(j, kp)]
                    for kq in range(KBF // 2):
                        del qwb[(j, kq)]

                emit_wq(0, dma_tiles=w0q)
                for j in range(1, NT):
                    emit_wq(j)
                    emit_mms(j - 1)
                emit_mms(NT - 1)

    nc.compile()
    return nc


_PROGRAM = None


def _get_program():
    global _PROGRAM
    if _PROGRAM is None:
        _PROGRAM = build_bass()
    return _PROGRAM


def make_in_maps(x, W, b):
    """Shard full inputs into the 8 per-core input dicts."""
    x = np.ascontiguousarray(x, dtype=np.float32).reshape(T_FULL, D_IN)
    xb = x.astype(ml_dtypes.bfloat16)
    W = np.ascontiguousarray(W, dtype=np.float32)
    b = np.ascontiguousarray(b, dtype=np.float32).reshape(1, D_OUT)
    wt = np.ascontiguousarray(W.T)  # [in, out]
    in_maps = []
    for c in range(N_CORES):
        blks = [(c + j) % N_CORES for j in range(NT)]
        wtb = np.concatenate(
            [wt[:, blk * 512 : (blk + 1) * 512] for blk in blks], axis=0
        )
        brot = np.concatenate(
            [b[:, blk * 512 : (blk + 1) * 512] for blk in blks], axis=1
        )
        in_maps.append(
            {
                "xs": np.ascontiguousarray(xb[c * T : (c + 1) * T]),
                "wtb": np.ascontiguousarray(wtb),
                "bias": np.ascontiguousarray(brot),
            }
        )
    return in_maps


def kernel(x, W, b, trace=False, tmpdir=None):
    nc = _get_program()
    res = run_bass_kernel_spmd(
        nc,
        make_in_maps(x, W, b),
        core_ids=list(range(N_CORES)),
        trace=trace,
        tmpdir=tmpdir,
    )
    out = np.empty((T_FULL, D_OUT), dtype=np.float32)
    for c in range(N_CORES):
        oc = res.results[c]["out"].reshape(T, NT, 512)
        for j in range(NT):
            blk = (c + j) % N_CORES
            out[c * T : (c + 1) * T, blk * 512 : (blk + 1) * 512] = oc[:, j]
    out = out.reshape(B, S, D_OUT)
    if trace:
        kernel.last_results = res
    return out


# revision 11
# speedup vs baseline: 1.1235x; 1.1235x over previous
"""BitLinear (BitNet b1.58-style) kernel for Trainium2, 8-core SPMD.

Reference computation (fp32):
    scale_w = max(mean(|W|), EPS)                       # scalar over all of W
    dq_w    = clip(round(W / scale_w), -1, 1) * scale_w # ternary weight
    amax_t  = max(max_j |x[t, j]|, EPS)                 # per token
    s_t     = 127 / amax_t
    dq_x    = round(x * s_t) / s_t                      # 8-bit absmax act quant
    out     = dq_x @ dq_w.T + b

Device strategy (data-parallel over tokens, fp8 DoubleRow matmul):
  * x is flattened to [8192, 4096] tokens, sharded 8 ways (1024/core) and
    shipped as bf16 (the 2e-2 tolerance dwarfs bf16 rounding; verified by
    exact simulation of this kernel's arithmetic: rel-err 0.0155, and the
    on-device run reproduces the simulated maxdiff bit-for-bit).
  * q_x = round(x*s) in [-127,127].  The first KF8=24 k-tiles are stored
    as fp8e4 (e4m3 RNE - the only lossy step) and their matmuls run pairs
    of k-tiles with perf_mode=DoubleRow (2 fp8 weights/PE cell, K=256 per
    instruction) at ~1.44x bf16 throughput.  The last 8 k-tiles stay
    exact bf16 to buy error margin.  q_w in {-1,0,1} is exact in both.
  * x is quantized token-major (per-token scale = per-partition ACT scale
    with a fused +C RNE round), then moved to feature-major by xbar
    DMA-transpose (2-byte dtype) + a GPSIMD cast copy.  No PE transposes.
  * W ships as W.T, column-blocked in per-core rotated order, so each
    core's first block doubles as its 1/8 shard for the |W| mean.  A
    dummy 4-byte AllReduce is issued at t=0 to absorb the cross-core
    startup-skew barrier so the real mean AllReduce completes quickly.
  * Engine plan: DVE = mean reduces, amax chain, W quant clips, evict
    scales; ACT = x round, W scale pass + its HWDGE ring carries W loads
    and output stores; sync ring = x loads + xbar transposes; GPSIMD =
    collectives, fp8/bf16 cast copies, bias adds, 2 W-quarter loads.
  * Matmul order: block 0 pair-inner (so it can start while phase A is
    still producing token tiles); later blocks token-outer with DR/bf16
    interleaved (hides the DoubleRow LDWEIGHTS under bf16 matmuls) and
    eviction immediately after each token tile's accumulation.
"""

import numpy as np
import ml_dtypes

from concourse import bacc, bass_isa, mybir, tile
from concourse.bass_utils import run_bass_kernel_spmd

F32 = mybir.dt.float32
BF16 = mybir.dt.bfloat16
FP8 = mybir.dt.float8e4
AX = mybir.AxisListType
OP = mybir.AluOpType
AF = mybir.ActivationFunctionType
PM = mybir.MatmulPerfMode

EPS = 1e-6
QMAX = 127.0
C_MAGIC = 1.5 * 2.0**23  # fp32 RNE rounding constant

N_CORES = 8
B, S, D_IN, D_OUT = 4, 2048, 4096, 4096
T_FULL = B * S
T = T_FULL // N_CORES  # 1024 tokens per core
KT = D_IN // 128  # 32 contraction k-tiles
KF8 = 24  # k-tiles carried in fp8 (12 DoubleRow pairs)
KBF = KT - KF8  # k-tiles carried in bf16 (exact)
NP8 = KF8 // 2  # 12 fp8 pairs
NT = D_OUT // 512  # 8 output column blocks
MT = T // 128  # 8 token tiles


def build_bass():
    nc = bacc.Bacc(None)
    xs_d = nc.dram_tensor("xs", [T, D_IN], BF16, kind="ExternalInput")
    wtb_d = nc.dram_tensor("wtb", [NT * D_IN, 512], F32, kind="ExternalInput")
    b_d = nc.dram_tensor("bias", [1, D_OUT], F32, kind="ExternalInput")
    out_d = nc.dram_tensor("out", [T, D_OUT], F32, kind="ExternalOutput")

    def wslice(j, qi):
        # quarter qi of column-block j: [1024, 512] of wtb, k-major
        r0 = (j * 4 + qi) * 1024
        return wtb_d[r0 : r0 + 1024, :].rearrange("(a p) c -> p a c", p=128)

    with tile.TileContext(nc) as tc:
        with (
            tc.tile_pool(name="persist", bufs=1) as persist,
            tc.tile_pool(name="small", bufs=2) as small,
            tc.tile_pool(name="wpipe", bufs=4) as wpipe,
            tc.tile_pool(name="dram", bufs=1, space="DRAM") as dram,
        ):
            # persistent state
            qxT8 = persist.tile([128, KF8, T], FP8)
            qxTb = persist.tile([128, KBF, T], BF16)
            bb = persist.tile([128, D_OUT], F32)
            amax_e = persist.tile([128, MT], F32)  # max(amax, EPS) per token
            s_all = persist.tile([128, MT], F32)  # 127/amax per token
            c_all = persist.tile([128, MT], F32)  # amax*scale_w/127 per token
            scw = persist.tile([128, 1], F32)
            inv_w = persist.tile([128, 1], F32)

            # bias: load row 0 (sync), broadcast across partitions (gpsimd)
            nc.sync.dma_start(bb[0:1, :], b_d[:])
            nc.gpsimd.partition_broadcast(bb[:], bb[0:1, :], channels=128)

            # ---- |W| mean over this core's first column block ------------
            # eighth-sized loads on the ACT HWDGE ring, reduces on DVE
            wsum_p = small.tile([128, 8], F32)
            with tc.tile_pool(name="meanpipe", bufs=2) as meanpipe:
                for e in range(8):
                    mtl = meanpipe.tile([128, 4, 512], F32, tag="m_in")
                    nc.scalar.dma_start(
                        mtl[:],
                        wtb_d[e * 512 : (e + 1) * 512, :].rearrange(
                            "(a p) c -> p a c", p=128
                        ),
                    )
                    nc.vector.tensor_reduce(
                        out=wsum_p[:, e : e + 1],
                        in_=mtl[:],
                        axis=AX.XY,
                        op=OP.add,
                        apply_absolute_value=True,
                    )
            wsum1 = small.tile([128, 1], F32)
            nc.vector.tensor_reduce(out=wsum1[:], in_=wsum_p[:], axis=AX.X, op=OP.add)
            wsum_all = small.tile([128, 1], F32)
            nc.gpsimd.partition_all_reduce(
                wsum_all[:], wsum1[:], channels=128, reduce_op=bass_isa.ReduceOp.add
            )
            cc_in = dram.tile([1, 1], F32)
            cc_out = dram.tile([1, 1], F32, addr_space="Shared")
            nc.gpsimd.dma_start(cc_in[:], wsum_all[0:1, 0:1])
            nc.gpsimd.collective_compute(
                "AllReduce",
                OP.add,
                replica_groups=[list(range(N_CORES))],
                ins=[cc_in[:]],
                outs=[cc_out[:]],
            )

            # block-0 W quarters: 0/1 on the ACT ring (after the mean
            # eighths), 2/3 via GPSIMD SWDGE (after the collective issue)
            w0q = []
            for qi in range(4):
                wq = wpipe.tile([128, 4096], F32, tag="w_in", name="wq")
                eng = nc.scalar if qi < 2 else nc.gpsimd
                eng.dma_start(wq[:], wslice(0, qi))
                w0q.append(wq)

            # ---- phase A: x quant (token-major) + xbar transpose ---------
            with (
                tc.tile_pool(name="xpipe", bufs=2) as xpipe,
                tc.tile_pool(name="tpipe", bufs=2) as tpipe,
            ):
                for m in range(MT):
                    xtl = xpipe.tile([128, D_IN], BF16, tag="x_in")
                    nc.sync.dma_start(xtl[:], xs_d[m * 128 : (m + 1) * 128, :])
                    amax = xpipe.tile([128, 1], F32, tag="amax")
                    nc.vector.tensor_reduce(
                        out=amax[:],
                        in_=xtl[:],
                        axis=AX.X,
                        op=OP.max,
                        apply_absolute_value=True,
                    )
                    nc.vector.tensor_scalar(
                        amax_e[:, m : m + 1], amax[:], EPS, None, op0=OP.max
                    )
                    rec = xpipe.tile([128, 1], F32, tag="rec")
                    nc.vector.reciprocal(rec[:], amax_e[:, m : m + 1])
                    nc.vector.tensor_scalar(
                        s_all[:, m : m + 1], rec[:], QMAX, None, op0=OP.mult
                    )
                    # q = round(x*s) on ACT: v = x*s + C (RNE integer),
                    # then -C -> bf16 on DVE (q ints are bf16-exact)
                    qxc = xpipe.tile([128, D_IN], BF16, tag="qxc")
                    for h in range(2):
                        hs = slice(h * 2048, (h + 1) * 2048)
                        qq = xpipe.tile([128, 2048], F32, tag="qq")
                        nc.scalar.activation(
                            qq[:], xtl[:, hs], AF.Copy,
                            bias=C_MAGIC, scale=s_all[:, m : m + 1],
                        )
                        nc.vector.tensor_scalar(
                            qxc[:, hs], qq[:], C_MAGIC, None, op0=OP.subtract
                        )
                    # feature-major via xbar transpose on the sync ring,
                    # then cast copies on GPSIMD
                    t8 = tpipe.tile([128, KF8, 128], BF16, tag="t8")
                    nc.sync.dma_start_transpose(t8[:], qxc[:, 0 : KF8 * 128])
                    nc.vector.tensor_copy(qxT8[:, :, m * 128 : (m + 1) * 128], t8[:])
                    tb = tpipe.tile([128, KBF, 128], BF16, tag="tb")
                    nc.sync.dma_start_transpose(tb[:], qxc[:, KF8 * 128 :])
                    nc.vector.tensor_copy(qxTb[:, :, m * 128 : (m + 1) * 128], tb[:])

            # AllReduce readback (gpsimd, after the cast copies)
            tot = small.tile([1, 1], F32)
            nc.gpsimd.dma_start(tot[:], cc_out[:])
            tot_b = small.tile([128, 1], F32)
            nc.gpsimd.partition_broadcast(tot_b[:], tot[:], channels=128)

            # scale_w = max(total/(D_IN*D_OUT), EPS); inv_w = 1/scale_w
            nc.vector.tensor_scalar(
                scw[:], tot_b[:], 1.0 / (D_IN * D_OUT), EPS, op0=OP.mult, op1=OP.max
            )
            nc.vector.reciprocal(inv_w[:], scw[:])
            nc.vector.tensor_scalar(
                c_all[:], amax_e[:], scw[:, 0:1], 1.0 / QMAX, op0=OP.mult, op1=OP.mult
            )

            # ---- phase B: quantize W, matmul, evict ----------------------
            # Software pipelined: W-quant for block j+1 is emitted before
            # the matmuls of block j so its ACT/DVE work runs under them.
            qw8 = {}  # (j, kp) -> fp8 pair tile, kp in [0, NP8)
            qwb = {}  # (j, kq) -> bf16 pair tile, kq in [0, KBF//2)

            with (
                tc.tile_pool(name="qwpipe", bufs=1) as qwpipe,
                tc.tile_pool(name="opipe", bufs=5) as opipe,
                tc.tile_pool(name="psumB", bufs=1, space="PSUM") as psumB,
            ):

                def emit_wq(j, dma_tiles=()):
                    # quarter-granular: big flat 2D ops keep the DVE cost
                    # well under the PE time per block
                    for qi in range(4):
                        if qi < len(dma_tiles):
                            wq = dma_tiles[qi]
                        else:
                            wq = wpipe.tile([128, 4096], F32, tag="w_in", name="wq")
                            nc.scalar.dma_start(wq[:], wslice(j, qi))
                        # u = W*inv_w on ACT (separate op: the +C round
                        # then sees fl(W/scale) like the ref)
                        nc.scalar.activation(
                            wq[:], wq[:], AF.Copy, bias=0.0, scale=inv_w[:, 0:1]
                        )
                        # +C RNE round and clip in the C domain
                        nc.vector.tensor_scalar(
                            wq[:], wq[:], C_MAGIC, C_MAGIC + 1.0,
                            op0=OP.add, op1=OP.min,
                        )
                        if qi < 3:
                            qt = qwpipe.tile(
                                [128, 4096], FP8, tag=f"q8_{j % 2}", bufs=3
                            )
                            qw8[(j, qi)] = qt
                        else:
                            qt = qwpipe.tile(
                                [128, 4096], BF16, tag=f"qb_{j % 2}", bufs=1
                            )
                            qwb[j] = qt
                        nc.vector.tensor_scalar(
                            qt[:], wq[:], C_MAGIC - 1.0, C_MAGIC,
                            op0=OP.max, op1=OP.subtract,
                        )

                def mm_dr(ps, j, kp, m, start, stop=False):
                    qt = qw8[(j, kp // 4)]
                    pp = kp % 4
                    rhs = qt[:].rearrange("p (a c) -> p a c", a=8)[
                        :, 2 * pp : 2 * pp + 2, :
                    ]
                    nc.tensor.matmul(
                        ps[:],
                        qxT8[:, 2 * kp : 2 * kp + 2, m * 128 : (m + 1) * 128],
                        rhs,
                        start=start,
                        stop=stop,
                        perf_mode=PM.DoubleRow,
                    )

                def mm_bf(ps, j, kb, m, stop, start=False):
                    rhs = qwb[j][:].rearrange("p (a c) -> p a c", a=8)[:, kb, :]
                    nc.tensor.matmul(
                        ps[:],
                        qxTb[:, kb, m * 128 : (m + 1) * 128],
                        rhs,
                        start=start,
                        stop=stop,
                    )

                def emit_evict(j, m, ps):
                    ot = opipe.tile([128, 512], F32, tag="o_scaled")
                    nc.vector.tensor_scalar(
                        ot[:], ps[:], c_all[:, m : m + 1], None, op0=OP.mult
                    )
                    ot2 = opipe.tile([128, 512], F32, tag="o_final")
                    nc.gpsimd.tensor_tensor(
                        ot2[:], ot[:], bb[:, j * 512 : (j + 1) * 512], op=OP.add
                    )
                    nc.scalar.dma_start(
                        out_d[m * 128 : (m + 1) * 128, j * 512 : (j + 1) * 512],
                        ot2[:],
                    )

                def emit_mms(j):
                    pss = [None] * MT
                    for m in reversed(range(MT)):
                        pss[m] = psumB.tile([128, 512], F32, tag=f"mm{m}", name="ps")
                    if j == 0:
                        # pair-inner: consumes qw pairs at production rate
                        # and token tiles as phase A finishes them
                        for kp in range(NP8):
                            for m in range(MT):
                                mm_dr(pss[m], j, kp, m, start=(kp == 0))
                        for kb in range(KBF):
                            for m in range(MT):
                                mm_bf(pss[m], j, kb, m, stop=(kb == KBF - 1))
                        for m in range(MT):
                            emit_evict(j, m, pss[m])
                    else:
                        # token-outer with DR/bf16 interleave (bf16 FWL
                        # loads hide the DoubleRow LDWEIGHTS), eviction
                        # right after each token tile completes
                        for m in range(MT):
                            order = []
                            for k in range(KBF):
                                order += [("dr", k), ("bf", k)]
                            order += [("dr", k) for k in range(KBF, NP8)]
                            for i, (kind, k) in enumerate(order):
                                first = i == 0
                                last = i == len(order) - 1
                                if kind == "dr":
                                    mm_dr(pss[m], j, k, m, start=first, stop=last)
                                else:
                                    mm_bf(pss[m], j, k, m, stop=last, start=first)
                            emit_evict(j, m, pss[m])
                    for qi in range(3):
                        del qw8[(j, qi)]
                    del qwb[j]

                emit_wq(0, dma_tiles=w0q)
                for j in range(1, NT):
                    emit_wq(j)
                    emit_mms(j - 1)
                emit_mms(NT - 1)

    nc.compile()
    return nc


_PROGRAM = None


def _get_program():
    global _PROGRAM
    if _PROGRAM is None:
        _PROGRAM = build_bass()
    return _PROGRAM


def make_in_maps(x, W, b):
    """Shard full inputs into the 8 per-core input dicts."""
    x = np.ascontiguousarray(x, dtype=np.float32).reshape(T_FULL, D_IN)
    xb = x.astype(ml_dtypes.bfloat16)
    W = np.ascontiguousarray(W, dtype=np.float32)
    b = np.ascontiguousarray(b, dtype=np.float32).reshape(1, D_OUT)
    wt = np.ascontiguousarray(W.T)  # [in, out]
    in_maps = []
    for c in range(N_CORES):
        blks = [(c + j) % N_CORES for j in range(NT)]
        wtb = np.concatenate(
            [wt[:, blk * 512 : (blk + 1) * 512] for blk in blks], axis=0
        )
        brot = np.concatenate(
            [b[:, blk * 512 : (blk + 1) * 512] for blk in blks], axis=1
        )
        in_maps.append(
            {
                "xs": np.ascontiguousarray(xb[c * T : (c + 1) * T]),
                "wtb": np.ascontiguousarray(wtb),
                "bias": np.ascontiguousarray(brot),
            }
        )
    return in_maps


def kernel(x, W, b, trace=False, tmpdir=None):
    nc = _get_program()
    res = run_bass_kernel_spmd(
        nc,
        make_in_maps(x, W, b),
        core_ids=list(range(N_CORES)),
        trace=trace,
        tmpdir=tmpdir,
    )
    out = np.empty((T_FULL, D_OUT), dtype=np.float32)
    for c in range(N_CORES):
        oc = res.results[c]["out"].reshape(T, NT, 512)
        for j in range(NT):
            blk = (c + j) % N_CORES
            out[c * T : (c + 1) * T, blk * 512 : (blk + 1) * 512] = oc[:, j]
    out = out.reshape(B, S, D_OUT)
    if trace:
        kernel.last_results = res
    return out
